# revision 18
# baseline (speedup 1.0000x reference)
"""Debayer 3x3 kernel for Trainium2 (Bass/Tile), batch-sharded over 8 NeuronCores.

Reference semantics: 1->5 channel 3x3 conv (identity, plus-4, diag-4,
horiz-2, vert-2) over an edge-padded Bayer frame, then per-2x2-parity
channel select into RGB.

Per-pixel, with q = x/4:
  SQ = q[left]+q[right]  (= H/4)     VQ = q[up]+q[down]  (= V/4)
  c0 = x = 4q   c1 = SQ+VQ   c2 = SQ[up]+SQ[down]   c3 = 2*SQ   c4 = 2*VQ
RGB parity table (row parity, col parity):
  R: (e,e)=c0 (e,o)=c3 (o,e)=c4 (o,o)=c2
  G: (e,e)=c1 (e,o)=c0 (o,e)=c0 (o,o)=c1
  B: (e,e)=c2 (e,o)=c4 (o,e)=c3 (o,o)=c0

Device layout: the host pre-tiles each padded 1090x1922 image into
128 partitions x 4 col-slices x (36 rows x 122 cols) patches:
  partition p = 32*q + b  (col-quarter q in 0..3, row-band b in 0..31)
  band b   -> image rows [34b, 34b+34)        (patch has +-1 halo rows)
  slice s  -> image cols [480q+120s, +120)    (patch has +-1 halo cols)
All stencil shifts are then free-dim AP offsets; parity classes are
stride-2 APs. 34 and 120 are even so parity phase is uniform across
partitions/slices.
"""

import numpy as np

H, W = 1088, 1920
NB = 32          # row bands per column-quarter
BH = 34          # output rows per band
NQ = 4           # column quarters
NS = 4           # col slices per patch
SW = 120         # output cols per slice
PR, PC = BH + 2, SW + 2   # patch rows/cols (with halo)

_NC_CACHE = {}
LAST_RESULTS = None


def _build(reps=1, *, no_compute=False, no_act=False, out_engine="sync",
           in_bufs=2, mid_bufs=2, out_bufs=2, vq_bufs=None):
    """Build the Bass module. reps>1 repeats the whole pipeline (bench only:
    amortizes per-dispatch overhead out of wall-clock measurements)."""
    key = (reps, no_compute, no_act, out_engine, in_bufs, mid_bufs, out_bufs,
           vq_bufs)
    if key in _NC_CACHE:
        return _NC_CACHE[key]
    import concourse.bacc as bacc
    import concourse.mybir as mybir
    import concourse.tile as tile
    from concourse._compat import get_trn_type

    f32 = mybir.dt.float32
    nc = bacc.Bacc(get_trn_type() or "TRN2", target_bir_lowering=False, debug=False)
    xin = nc.dram_tensor("xprep", [128, NS, PR, PC], f32, kind="ExternalInput")
    yout = nc.dram_tensor("yout", [3, 128, NS, BH, SW], f32, kind="ExternalOutput")
    # bench-only: earlier reps dump to internal scratch so no two reps write
    # the same DRAM (WAW races hang the exec unit)
    ydumps = [
        nc.dram_tensor(f"ydump{r}", [3, 128, NS, BH, SW], f32, kind="Internal")
        for r in range(reps - 1)
    ]

    # out-row/out-col parity slices (within [BH, SW] output tiles)
    E_, O_ = slice(0, BH, 2), slice(1, BH, 2)
    e_, o_ = slice(0, SW, 2), slice(1, SW, 2)
    # patch-row slice for out rows of given parity (out row i -> patch row i+1)
    pE, pO = slice(1, PR - 1, 2), slice(2, PR, 2)
    # patch-col slice for out cols of given parity (out col j -> patch col j+1)
    ce, co = slice(1, PC - 1, 2), slice(2, PC, 2)
    # SQ rows for diag channel: out row i needs patch rows i and i+2
    dE0, dE1 = slice(0, PR - 2, 2), slice(2, PR, 2)      # even out rows
    dO0, dO1 = slice(1, PR - 1, 2), slice(3, PR, 2)      # odd out rows

    with tile.TileContext(nc) as tc:
        with tc.tile_pool(name="pin", bufs=in_bufs) as pin, \
             tc.tile_pool(name="pmid", bufs=mid_bufs) as pmid, \
             tc.tile_pool(name="pout", bufs=out_bufs) as pout:

            dma_out = nc.scalar if out_engine == "scalar" else nc.sync

            def load(j):
                t = pin.tile([128, PR, PC], f32, tag="inp", name=f"inp{j}")
                nc.sync.dma_start(out=t[:], in_=xin[:, j % NS])
                return t

            cur = load(0)
            for j in range(NS * reps):
                k = j % NS
                r = j // NS
                ytgt = yout if r == reps - 1 else ydumps[r]
                nxt = load(j + 1) if j + 1 < NS * reps else None
                Q = cur
                R = pout.tile([128, BH, SW], f32, tag="r", name=f"r{k}")
                G = pout.tile([128, BH, SW], f32, tag="g", name=f"g{k}")
                B = pout.tile([128, BH, SW], f32, tag="b", name=f"b{k}")
                if no_compute:
                    # bench-only: DMA skeleton (touch input once so it's live)
                    nc.vector.tensor_copy(R[:, 0:1, 0:SW], Q[:, 0:1, 0:SW])
                    for ch, t in ((0, R), (1, G), (2, B)):
                        dma_out.dma_start(out=ytgt[ch, :, k], in_=t[:])
                    cur = nxt
                    continue
                # prescale in place: Q = x/4
                nc.vector.tensor_scalar_mul(Q[:], Q[:], 0.25)
                # SQ[p, r, j] = H/4 at patch row r, out col j
                SQ = pmid.tile([128, PR, SW], f32, tag="sq", name=f"sq{k}")
                nc.vector.tensor_add(SQ[:], Q[:, :, 0:SW], Q[:, :, 2:PC])
                # VQ[p, i, j] = V/4 at out row i, out col j
                VQ = pmid.tile([128, BH, SW], f32, tag="vq", name=f"vq{k}",
                               bufs=vq_bufs)
                nc.vector.tensor_add(VQ[:], Q[:, 0:PR - 2, 1:PC - 1], Q[:, 2:PR, 1:PC - 1])

                if no_act:
                    def act_mul(out, in_, s):
                        nc.vector.tensor_scalar_mul(out, in_, s)
                else:
                    act_mul = nc.scalar.mul
                # ---- R ----
                nc.vector.tensor_add(R[:, O_, o_], SQ[:, dO0, o_], SQ[:, dO1, o_])  # c2
                act_mul(R[:, E_, e_], Q[:, pE, ce], 4.0)                            # c0
                act_mul(R[:, E_, o_], SQ[:, pE, o_], 2.0)                           # c3
                act_mul(R[:, O_, e_], VQ[:, O_, e_], 2.0)                           # c4
                dma_out.dma_start(out=ytgt[0, :, k], in_=R[:])
                # ---- G ----
                nc.vector.tensor_add(G[:, E_, e_], SQ[:, pE, e_], VQ[:, E_, e_])    # c1
                nc.vector.tensor_add(G[:, O_, o_], SQ[:, pO, o_], VQ[:, O_, o_])    # c1
                act_mul(G[:, E_, o_], Q[:, pE, co], 4.0)                            # c0
                act_mul(G[:, O_, e_], Q[:, pO, ce], 4.0)                            # c0
                dma_out.dma_start(out=ytgt[1, :, k], in_=G[:])
                # ---- B ----
                nc.vector.tensor_add(B[:, E_, e_], SQ[:, dE0, e_], SQ[:, dE1, e_])  # c2
                act_mul(B[:, E_, o_], VQ[:, E_, o_], 2.0)                           # c4
                act_mul(B[:, O_, e_], SQ[:, pO, e_], 2.0)                           # c3
                act_mul(B[:, O_, o_], Q[:, pO, co], 4.0)                            # c0
                dma_out.dma_start(out=ytgt[2, :, k], in_=B[:])

                cur = nxt

    nc.compile()
    _NC_CACHE[key] = nc
    return nc


def _prep_inputs(x):
    """(B,1,1088,1920) -> (B,128,NS,PR,PC) patch layout (edge padded)."""
    Bn = x.shape[0]
    xpad = np.pad(x[:, 0], ((0, 0), (1, 1), (1, 1)), mode="edge")  # (B,1090,1922)
    xprep = np.empty((Bn, 128, NS, PR, PC), np.float32)
    st = xpad.strides
    for q in range(NQ):
        for s in range(NS):
            c0 = 480 * q + SW * s
            block = xpad[:, :, c0:c0 + PC]
            v = np.lib.stride_tricks.as_strided(
                block, shape=(Bn, NB, PR, PC),
                strides=(st[0], BH * st[1], st[1], st[2]))
            xprep[:, q * NB:(q + 1) * NB, s] = v
    return xprep


def _assemble(y):
    """(3,128,NS,BH,SW) -> (3,1088,1920)."""
    out = np.empty((3, H, W), np.float32)
    for q in range(NQ):
        rows = y[:, q * NB:(q + 1) * NB]          # (3,NB,NS,BH,SW)
        for s in range(NS):
            c0 = 480 * q + SW * s
            out[:, :, c0:c0 + SW] = rows[:, :, s].reshape(3, H, SW)
    return out


def kernel(x, kernels=None, index=None, **_unused):
    global LAST_RESULTS
    x = np.ascontiguousarray(np.asarray(x), dtype=np.float32)
    Bn = x.shape[0]
    xprep = _prep_inputs(x)
    nc = _build(in_bufs=3, vq_bufs=1)
    from concourse.bass_utils import run_bass_kernel_spmd
    in_maps = [{"xprep": xprep[i]} for i in range(Bn)]
    res = run_bass_kernel_spmd(nc, in_maps, core_ids=list(range(Bn)))
    LAST_RESULTS = res
    out = np.empty((Bn, 3, H, W), np.float32)
    for i in range(Bn):
        out[i] = _assemble(res.results[i]["yout"])
    return out
